# revision 10
# baseline (speedup 1.0000x reference)
"""Weighted-BCE loss kernel for Trainium2 (8 NeuronCores, SPMD data-parallel).

Reference math (torch-style BCELoss with class-balancing weights):
    n   = len(x), s = sum(gt)
    w0  = n / (2*(n-s)),  w1 = n / (2*s)
    L1  = max(log(x),     -100)
    L0  = max(log1p(-x),  -100)
    loss = mean( where(gt==0, w0, w1) * -(gt*L1 + (1-gt)*L0) )

Only ONE of log(x) / log(1-x) is needed per element (selected by gt), so
instead of two Ln passes we compute the selected operand in one shot:
    z = gt ? x : 1-x  =  1 - |x' - gt|,   x' = max(x, 2^-24)
(the clamp rides the op0 slot of the w-STT for free and guarantees
z >= 2^-24, so Ln never sees 0; vs the reference's -100 clamp this only
misvalues exact x==0 elements - ~1 in 16.7M, error ~5e-6 of the loss).

Global sums, all computed shard-locally (weights need only GLOBAL s):
    A  = sum(gt * Lz)        [DVE STT accum]  = sum_{gt=1} log x
    T  = sum(Lz)             [ACT accum, free on the Ln pass]
    W  = sum(x' - gt)        [free accum on the w-STT]
    Sx = sum(x)              [PE column-sum matmuls -> one PSUM bank]
    s  = Sx - W  (exact to ~1e-6 rel), so no separate sum(gt) pass.
    loss = -( A/(2s) + (T-A)/(2(n-s)) )

Engine balance per [128, 4096] tile (its two DMAs take ~12us, measured
rates: DVE STT 1.29ns/col, ACT pass 1.09ns/col, dtype-independent):
    DVE  w-STT 5.3us + A-STT 5.3us
    ACT  Abs(w) in-place 4.5us + Ln 4.5us + gt-DMA issue 0.7us
    PE   8 x [128,512] colsum matmuls (accumulate into PSUM)
    SP   x DMA
The A-STT is emitted one tile late so DVE always has the next tile's
independent w-STT queued ahead of the cross-engine Ln dependency, and
it consumes gt directly (gp pool holds 4 bufs to cover its lifetime).
First tile is small so compute starts ~4us after the queues open (a
4096-first-tile costs a 16us dead ramp); last tiles are small to
shrink the post-final-DMA drain chain.  Host gathers the [128, 3*NT]
accums + the [1, 512] PSUM colsum from all 8 cores and finishes the
(tiny) all-reduce + scalar math in float64.
"""

import numpy as np
from contextlib import ExitStack

import concourse.bass as bass
import concourse.bacc as bacc
import concourse.mybir as mybir
import concourse.tile as tile
from concourse.alu_op_type import AluOpType
from concourse.bass_utils import run_bass_kernel_spmd

N_TOTAL = 16777216
N_CORES = 8
PER_CORE = N_TOTAL // N_CORES   # 2097152
P = 128
FD = PER_CORE // P              # 16384 free elements per partition
TILE_SIZES = [1024, 2048, 4096, 4096, 3072, 1024, 512, 512]
assert sum(TILE_SIZES) == FD
NT = len(TILE_SIZES)
MM = 512                        # moving free-dim chunk for PE colsums
X_CLAMP = 5.9604645e-08         # 2^-24: keeps z = 1-|x'-gt| >= 2^-24
LOG_CLAMP = -100.0

# Optional instrumentation knobs for a driver script (harness never sets them).
TRACE = False
LAST_RESULTS = None

_NC_CACHE = None


def _build():
    f32 = mybir.dt.float32
    i32 = mybir.dt.int32
    Ln = mybir.ActivationFunctionType.Ln
    Abs = mybir.ActivationFunctionType.Abs

    nc = bacc.Bacc("TRN2")
    x_in = nc.declare_dram_parameter("x", [P, FD], f32, isOutput=False)
    g_in = nc.declare_dram_parameter("gt", [P, FD], i32, isOutput=False)
    # packed accum output: columns [A | T | W], NT each
    out_all = nc.declare_dram_parameter("out_all", [P, 3 * NT], f32, isOutput=True)
    sum_x = nc.declare_dram_parameter("sum_x", [1, MM], f32, isOutput=True)

    n_mm = sum(t // MM for t in TILE_SIZES)

    with tile.TileContext(nc) as tc, ExitStack() as ctx:
        xp = ctx.enter_context(tc.tile_pool(name="xp", bufs=2))
        gp = ctx.enter_context(tc.tile_pool(name="gp", bufs=4))
        wp = ctx.enter_context(tc.tile_pool(name="wp", bufs=2))
        lp = ctx.enter_context(tc.tile_pool(name="lp", bufs=3))
        jp = ctx.enter_context(tc.tile_pool(name="jp", bufs=1))
        accp = ctx.enter_context(tc.tile_pool(name="accp", bufs=1))
        pp = ctx.enter_context(tc.psum_pool(name="pp", bufs=1))

        accA = accp.tile([P, NT], f32)
        accT = accp.tile([P, NT], f32)
        accW = accp.tile([P, NT], f32)
        groups = [accA, accT, accW]

        ones = accp.tile([P, 1], f32)
        nc.gpsimd.memset(ones[:], 1.0)

        psum_t = pp.tile([1, MM], f32)

        def col(group, i):
            return groups[group][:, i : i + 1]

        def emit_A(i, lz, gt_t, tfd):
            junk_a = jp.tile([P, tfd], f32, tag="junk_a")
            nc.vector.scalar_tensor_tensor(
                junk_a[:], lz[:], LOG_CLAMP, gt_t[:],
                AluOpType.max, AluOpType.mult,
                accum_out=col(0, i),
            )

        pending_A = None  # (i, lz, gt_t, tfd): emitted one tile late
        mm_idx = 0
        off = 0
        for i, tfd in enumerate(TILE_SIZES):
            sl = slice(off, off + tfd)
            off += tfd
            xt = xp.tile([P, tfd], f32, tag="xt")
            gt_t = gp.tile([P, tfd], i32, tag="gt")
            # two HWDGE queues: x via SP(sync), gt via the ACT sequencer
            nc.sync.dma_start(xt[:], x_in[:, sl])
            nc.scalar.dma_start(gt_t[:], g_in[:, sl])

            # w = max(x, 2^-24) - gt in [-1, 1];  accum -> W
            wt = wp.tile([P, tfd], f32, tag="w")
            nc.vector.scalar_tensor_tensor(
                wt[:], xt[:], X_CLAMP, gt_t[:],
                AluOpType.max, AluOpType.subtract,
                accum_out=col(2, i),
            )
            # Sx: accumulate column sums of x into one PSUM bank (idle PE)
            for c in range(0, tfd, MM):
                nc.tensor.matmul(
                    psum_t[:], ones[:], xt[:, c : c + MM],
                    start=(mm_idx == 0), stop=(mm_idx == n_mm - 1),
                )
                mm_idx += 1
            # d = |w| in place (ACT), then Lz = Ln(1 - d), accum -> T
            nc.scalar.activation(wt[:], wt[:], Abs)
            lz = lp.tile([P, tfd], f32, tag="lz")
            nc.scalar.activation(
                lz[:], wt[:], Ln, bias=1.0, scale=-1.0,
                accum_out=col(1, i),
            )
            # A-STT for the PREVIOUS tile (keeps independent DVE work ahead
            # of the cross-engine Ln dependency)
            if pending_A is not None:
                emit_A(*pending_A)
            pending_A = (i, lz, gt_t, tfd)

        emit_A(*pending_A)

        for k, g in enumerate(groups):
            nc.sync.dma_start(out_all[:, k * NT : (k + 1) * NT], g[:])
        sum_x_sb = accp.tile([1, MM], f32)
        nc.scalar.copy(sum_x_sb[:], psum_t[:])
        nc.sync.dma_start(sum_x[:, :], sum_x_sb[:])

    nc.compile()
    return nc


def get_nc():
    global _NC_CACHE
    if _NC_CACHE is None:
        _NC_CACHE = _build()
    return _NC_CACHE


def make_in_maps(x, gt):
    x = np.ascontiguousarray(np.asarray(x, dtype=np.float32).reshape(-1))
    gt = np.ascontiguousarray(np.asarray(gt, dtype=np.int32).reshape(-1))
    assert x.shape == (N_TOTAL,) and gt.shape == (N_TOTAL,)
    in_maps = []
    for c in range(N_CORES):
        sl = slice(c * PER_CORE, (c + 1) * PER_CORE)
        in_maps.append({
            "x": x[sl].reshape(P, FD),
            "gt": gt[sl].reshape(P, FD),
        })
    return in_maps


def combine(results):
    """All-reduce the per-core partial sums and finish the loss formula."""
    A = T = S = 0.0
    for r in results:
        o = r["out_all"].astype(np.float64)
        A += o[:, 0 * NT : 1 * NT].sum()
        T += o[:, 1 * NT : 2 * NT].sum()
        W = o[:, 2 * NT : 3 * NT].sum()
        Sx = r["sum_x"].astype(np.float64).sum()
        S += Sx - W                      # sum(gt) for this core
    n = float(N_TOTAL)
    result = -(A / (2.0 * S) + (T - A) / (2.0 * (n - S)))
    return np.array(result, dtype=np.float32)


def kernel(x, gt):
    global LAST_RESULTS
    nc = get_nc()
    in_maps = make_in_maps(x, gt)
    br = run_bass_kernel_spmd(nc, in_maps, list(range(N_CORES)))
    LAST_RESULTS = br
    return combine(br.results)


# revision 13
# speedup vs baseline: 1.0309x; 1.0309x over previous
"""Weighted-BCE loss kernel for Trainium2 (8 NeuronCores, SPMD data-parallel).

Reference math (torch-style BCELoss with class-balancing weights):
    n   = len(x), s = sum(gt)
    w0  = n / (2*(n-s)),  w1 = n / (2*s)
    L1  = max(log(x),     -100)
    L0  = max(log1p(-x),  -100)
    loss = mean( where(gt==0, w0, w1) * -(gt*L1 + (1-gt)*L0) )

Only ONE of log(x) / log(1-x) is needed per element (selected by gt), so
instead of two Ln passes we compute the selected operand in one shot:
    z = gt ? x : 1-x  =  1 - |x' - gt|,   x' = max(x, 2^-24)
(the clamp rides the op0 slot of the w-STT for free and guarantees
z >= 2^-24, so Ln never sees 0; vs the reference's -100 clamp this only
misvalues exact x==0 elements - ~1 in 16.7M, error ~5e-6 of the loss).

Global sums, all computed shard-locally (weights need only GLOBAL s):
    A  = sum(gt * Lz)        [DVE STT accum]  = sum_{gt=1} log x
    T  = sum(Lz)             [ACT accum, free on the Ln pass]
    W  = sum(x' - gt)        [free accum on the w-STT]
    Sx = sum(x)              [PE column-sum matmuls -> one PSUM bank]
    s  = Sx - W  (exact to ~1e-6 rel), so no separate sum(gt) pass.
    loss = -( A/(2s) + (T-A)/(2(n-s)) )

Engine balance per [128, 4096] tile (its two DMAs take ~12us, measured
rates: DVE STT 1.29ns/col, ACT pass 1.09ns/col, dtype-independent):
    DVE  w-STT 5.3us + A-STT 5.3us
    ACT  Abs(w) in-place 4.5us + Ln 4.5us + gt-DMA issue 0.7us
    PE   8 x [128,512] colsum matmuls (accumulate into PSUM)
    SP   x DMA
The A-STT is emitted one tile late so DVE always has the next tile's
independent w-STT queued ahead of the cross-engine Ln dependency, and
it consumes gt directly (gp pool holds 4 bufs to cover its lifetime).
First tile is small so compute starts ~4us after the queues open (a
4096-first-tile costs a 16us dead ramp); last tiles are small to
shrink the post-final-DMA drain chain.  Host gathers the [128, 3*NT]
accums + the [1, 512] PSUM colsum from all 8 cores and finishes the
(tiny) all-reduce + scalar math in float64.
"""

import numpy as np
from contextlib import ExitStack

import concourse.bass as bass
import concourse.bacc as bacc
import concourse.mybir as mybir
import concourse.tile as tile
from concourse.alu_op_type import AluOpType
from concourse.bass_utils import run_bass_kernel_spmd

N_TOTAL = 16777216
N_CORES = 8
PER_CORE = N_TOTAL // N_CORES   # 2097152
P = 128
FD = PER_CORE // P              # 16384 free elements per partition
TILE_SIZES = [1024, 1024, 2048, 2048, 2048, 2048, 2048, 2048, 1024, 512, 512]
assert sum(TILE_SIZES) == FD
NT = len(TILE_SIZES)
MM = 512                        # moving free-dim chunk for PE colsums
X_CLAMP = 5.9604645e-08         # 2^-24: keeps z = 1-|x'-gt| >= 2^-24
LOG_CLAMP = -100.0

# Optional instrumentation knobs for a driver script (harness never sets them).
TRACE = False
LAST_RESULTS = None

_NC_CACHE = None


def _build():
    f32 = mybir.dt.float32
    i32 = mybir.dt.int32
    Ln = mybir.ActivationFunctionType.Ln
    Abs = mybir.ActivationFunctionType.Abs

    nc = bacc.Bacc("TRN2")
    x_in = nc.declare_dram_parameter("x", [P, FD], f32, isOutput=False)
    g_in = nc.declare_dram_parameter("gt", [P, FD], i32, isOutput=False)
    # packed accum output: columns [A | T | W], NT each
    out_all = nc.declare_dram_parameter("out_all", [P, 3 * NT], f32, isOutput=True)
    sum_x = nc.declare_dram_parameter("sum_x", [1, MM], f32, isOutput=True)

    n_mm = sum(t // MM for t in TILE_SIZES)

    with tile.TileContext(nc) as tc, ExitStack() as ctx:
        xp = ctx.enter_context(tc.tile_pool(name="xp", bufs=4))
        gp = ctx.enter_context(tc.tile_pool(name="gp", bufs=6))
        wp = ctx.enter_context(tc.tile_pool(name="wp", bufs=3))
        lp = ctx.enter_context(tc.tile_pool(name="lp", bufs=4))
        jp = ctx.enter_context(tc.tile_pool(name="jp", bufs=2))
        accp = ctx.enter_context(tc.tile_pool(name="accp", bufs=1))
        pp = ctx.enter_context(tc.psum_pool(name="pp", bufs=1))

        accA = accp.tile([P, NT], f32)
        accT = accp.tile([P, NT], f32)
        accW = accp.tile([P, NT], f32)
        groups = [accA, accT, accW]

        ones = accp.tile([P, 1], f32)
        nc.gpsimd.memset(ones[:], 1.0)

        # dummy Ln: forces the natural_log act-table (contains abs/ln/copy)
        # to load during the preamble instead of mid-pipeline
        warm = accp.tile([P, 1], f32)
        nc.scalar.activation(warm[:], ones[:], Ln)

        psum_t = pp.tile([1, MM], f32)

        def col(group, i):
            return groups[group][:, i : i + 1]

        def emit_A(i, lz, gt_t, tfd):
            junk_a = jp.tile([P, tfd], f32, tag="junk_a")
            nc.vector.scalar_tensor_tensor(
                junk_a[:], lz[:], LOG_CLAMP, gt_t[:],
                AluOpType.max, AluOpType.mult,
                accum_out=col(0, i),
            )

        pending_A = None  # (i, lz, gt_t, tfd): emitted one tile late
        mm_idx = 0
        off = 0
        for i, tfd in enumerate(TILE_SIZES):
            sl = slice(off, off + tfd)
            off += tfd
            xt = xp.tile([P, tfd], f32, tag="xt")
            gt_t = gp.tile([P, tfd], i32, tag="gt")
            # two DMA queues: x via SP(sync) HWDGE, gt via the otherwise-idle
            # Pool/GpSimd sequencer (keeps ACT's in-order stream free of
            # dma_start stalls)
            nc.sync.dma_start(xt[:], x_in[:, sl])
            nc.gpsimd.dma_start(gt_t[:], g_in[:, sl])

            # w = max(x, 2^-24) - gt in [-1, 1];  accum -> W
            wt = wp.tile([P, tfd], f32, tag="w")
            nc.vector.scalar_tensor_tensor(
                wt[:], xt[:], X_CLAMP, gt_t[:],
                AluOpType.max, AluOpType.subtract,
                accum_out=col(2, i),
            )
            # Sx: accumulate column sums of x into one PSUM bank (idle PE)
            for c in range(0, tfd, MM):
                nc.tensor.matmul(
                    psum_t[:], ones[:], xt[:, c : c + MM],
                    start=(mm_idx == 0), stop=(mm_idx == n_mm - 1),
                )
                mm_idx += 1
            # d = |w| in place (ACT), then Lz = Ln(1 - d), accum -> T
            nc.scalar.activation(wt[:], wt[:], Abs)
            lz = lp.tile([P, tfd], f32, tag="lz")
            nc.scalar.activation(
                lz[:], wt[:], Ln, bias=1.0, scale=-1.0,
                accum_out=col(1, i),
            )
            # A-STT for the PREVIOUS tile (keeps independent DVE work ahead
            # of the cross-engine Ln dependency)
            if pending_A is not None:
                emit_A(*pending_A)
            pending_A = (i, lz, gt_t, tfd)

        emit_A(*pending_A)

        for k, g in enumerate(groups):
            nc.sync.dma_start(out_all[:, k * NT : (k + 1) * NT], g[:])
        sum_x_sb = accp.tile([1, MM], f32)
        nc.scalar.copy(sum_x_sb[:], psum_t[:])
        nc.sync.dma_start(sum_x[:, :], sum_x_sb[:])

    nc.compile()
    return nc


def get_nc():
    global _NC_CACHE
    if _NC_CACHE is None:
        _NC_CACHE = _build()
    return _NC_CACHE


def make_in_maps(x, gt):
    x = np.ascontiguousarray(np.asarray(x, dtype=np.float32).reshape(-1))
    gt = np.ascontiguousarray(np.asarray(gt, dtype=np.int32).reshape(-1))
    assert x.shape == (N_TOTAL,) and gt.shape == (N_TOTAL,)
    in_maps = []
    for c in range(N_CORES):
        sl = slice(c * PER_CORE, (c + 1) * PER_CORE)
        in_maps.append({
            "x": x[sl].reshape(P, FD),
            "gt": gt[sl].reshape(P, FD),
        })
    return in_maps


def combine(results):
    """All-reduce the per-core partial sums and finish the loss formula."""
    A = T = S = 0.0
    for r in results:
        o = r["out_all"].astype(np.float64)
        A += o[:, 0 * NT : 1 * NT].sum()
        T += o[:, 1 * NT : 2 * NT].sum()
        W = o[:, 2 * NT : 3 * NT].sum()
        Sx = r["sum_x"].astype(np.float64).sum()
        S += Sx - W                      # sum(gt) for this core
    n = float(N_TOTAL)
    result = -(A / (2.0 * S) + (T - A) / (2.0 * (n - S)))
    return np.array(result, dtype=np.float32)


def kernel(x, gt):
    global LAST_RESULTS
    nc = get_nc()
    in_maps = make_in_maps(x, gt)
    br = run_bass_kernel_spmd(nc, in_maps, list(range(N_CORES)))
    LAST_RESULTS = br
    return combine(br.results)
